# revision 10
# baseline (speedup 1.0000x reference)
"""BCRNN layer (bidirectional convolutional RNN) on 8 Trainium2 NeuronCores.

Problem: nb=1, nc=2, nt=12, nx=160, ny=160, hid=64, K=3, reflect padding,
complex conv decomposed into real convs, modReLU activation, forward +
backward temporal scans, output = sum of the two scans stacked (r, i).

Sharding: spatial rows (nx) split 8 ways (20 rows/core).  The temporal scan
runs locally per core; the 1-row conv halo of the recurrent state is
exchanged each step with an AllGather of boundary rows + per-core indirect
gather (per-core routing lives in an index input tensor so the single SPMD
program stays uniform; global-edge reflect rows are sourced the same way).

Compute strategy per conv: 9 shifted fp32r matmuls (K=128 = stacked
real/imag channels, M=128 = stacked real/imag outputs) accumulated in PSUM;
i2h conv is a single K=36 matmul over a host-built im2col of the tiny
2-channel input.  modReLU magnitude pairing (zr^2 + zi^2 across partition
halves) is done with one extra 0/1-matrix matmul.
"""
import numpy as np

import concourse.bass as bass
import concourse.bacc as bacc
import concourse.tile as tile
import concourse.mybir as mybir
from concourse.bass_utils import run_bass_kernel_spmd

P = 128
NC_CORES = 8
NT = 12
NX, NY = 160, 160
HID = 64
CIN = 2
ROWS = NX // NC_CORES      # 20 owned rows per core
TR = ROWS + 2              # 22 tile rows (with halo)
YP = NY + 2                # 162 padded cols
NSLOT = 8                  # cc slots: 4 fwd + 4 bwd boundary rows

f32 = mybir.dt.float32
f32r = mybir.dt.float32r
f16 = mybir.dt.float16
i32 = mybir.dt.int32
AF = mybir.ActivationFunctionType
ALU = mybir.AluOpType

# conv groups (r0, nrows) over owned rows; scan order: boundary groups first
SCAN_GROUPS = [(0, 3), (17, 3), (3, 3), (6, 3), (9, 3), (12, 3), (15, 2)]
PRE_GROUPS = [(0, 3), (3, 3), (6, 3), (9, 3), (12, 3), (15, 3), (18, 2)]
CC_ROWS = [1, 2, 19, 20]   # contributed tile rows per direction

_CACHED = {}
TRACE = False
LAST = {}


def _build():
    import os
    STAGE = int(os.environ.get("STAGE", "4"))
    if "nc" in _CACHED:
        return _CACHED["nc"]
    nc = bacc.Bacc(None, target_bir_lowering=False, debug=False,
                   num_devices=NC_CORES)

    # ---- I/O ----
    iter_il = nc.dram_tensor("iter_il", [NT, P, TR, YP], f16, kind="ExternalInput")
    im2col = nc.dram_tensor("im2col", [NT, 36, ROWS, NY], f16, kind="ExternalInput")
    wh2h = nc.dram_tensor("wh2h", [9, P, P], f16, kind="ExternalInput")
    wih = nc.dram_tensor("wih", [9, P, P], f16, kind="ExternalInput")
    wi2h = nc.dram_tensor("wi2h", [36, P], f16, kind="ExternalInput")
    pmat = nc.dram_tensor("pmat", [P, P], f16, kind="ExternalInput")
    b_pre = nc.dram_tensor("b_pre", [P, 1], f32, kind="ExternalInput")
    b_mod = nc.dram_tensor("b_mod", [P, 1], f32, kind="ExternalInput")
    hidx = nc.dram_tensor("hidx", [P, 4], i32, kind="ExternalInput")
    out = nc.dram_tensor("out", [NT, P, ROWS, NY], f32, kind="ExternalOutput")

    # ---- internal DRAM ----
    predr = nc.dram_tensor("predr", [NT, P, ROWS, NY], f32)
    sav = nc.dram_tensor("sav", [NT, P, ROWS, NY], f16)

    with tile.TileContext(nc) as tc:
        with (
            tc.tile_pool(name="wp", bufs=1) as wp,
            tc.tile_pool(name="dram", bufs=2, space="DRAM") as dram,
            tc.tile_pool(name="cps", bufs=4, space="PSUM") as cps,
            tc.tile_pool(name="mps", bufs=2, space="PSUM") as mps,
        ):
            # weights / constants
            wh = [wp.tile([P, P], f16, tag=f"wh{k}", name=f"wh{k}") for k in range(9)]
            wi = [wp.tile([P, P], f16, tag=f"wi{k}", name=f"wi{k}") for k in range(9)]
            for k in range(9):
                nc.sync.dma_start(out=wh[k][:], in_=wh2h[k])
                nc.sync.dma_start(out=wi[k][:], in_=wih[k])
            w36 = wp.tile([36, P], f16, tag="w36")
            nc.sync.dma_start(out=w36[:], in_=wi2h[:])
            pm = wp.tile([P, P], f16, tag="pm")
            nc.sync.dma_start(out=pm[:], in_=pmat[:])
            bp = wp.tile([P, 1], f32, tag="bp")
            nc.sync.dma_start(out=bp[:], in_=b_pre[:])
            bm = wp.tile([P, 1], f32, tag="bm")
            nc.sync.dma_start(out=bm[:], in_=b_mod[:])
            hix = wp.tile([P, 4], i32, tag="hix")
            nc.sync.dma_start(out=hix[:], in_=hidx[:])
            epst = wp.tile([P, 1], f32, tag="epst")
            nc.vector.memset(epst[:], 1e-6)

            # ---------------- phase 1: pre[t] = ih(iter) + i2h(input) + bias
            with (
                tc.tile_pool(name="itp", bufs=2) as itp,
                tc.tile_pool(name="icp", bufs=2) as icp,
                tc.tile_pool(name="prs", bufs=2) as prs,
            ):
                for t in range(NT):
                    it = itp.tile([P, TR, YP], f16, tag="it")
                    nc.sync.dma_start(out=it[:], in_=iter_il[t])
                    ic = icp.tile([36, ROWS, NY], f16, tag="ic")
                    nc.sync.dma_start(out=ic[:], in_=im2col[t])
                    stage = prs.tile([P, ROWS, NY], f32, tag="stage")
                    for r0, nr in PRE_GROUPS:
                        ps = cps.tile([P, 3, NY], f32, tag="cv")
                        for tap in range(9):
                            dxi, dyi = divmod(tap, 3)
                            nc.tensor.matmul(
                                out=ps[:, :nr, :],
                                lhsT=wi[tap][:],
                                rhs=it[:, r0 + dxi:r0 + dxi + nr, dyi:dyi + NY],
                                start=(tap == 0), stop=False,
                            )
                        nc.tensor.matmul(
                            out=ps[:, :nr, :],
                            lhsT=w36[:],
                            rhs=ic[:, r0:r0 + nr, :],
                            start=False, stop=True,
                        )
                        nc.scalar.activation(stage[:, r0:r0 + nr, :], ps[:, :nr, :],
                                             AF.Identity, bias=bp[:, 0:1], scale=1.0)
                    nc.sync.dma_start(out=predr[t], in_=stage[:])
                    if STAGE == 1:
                        nc.sync.dma_start(out=out[t], in_=stage[:])

            # ---------------- phase 2: bidirectional scan
            with (
                tc.tile_pool(name="hfp", bufs=2) as hfp,
                tc.tile_pool(name="hbp", bufs=2) as hbp,
                tc.tile_pool(name="pin", bufs=3) as pinp,
                tc.tile_pool(name="zp", bufs=2) as zp,
                tc.tile_pool(name="z2p", bufs=2) as z2p,
                tc.tile_pool(name="mgp", bufs=2) as mgp,
                tc.tile_pool(name="nmp", bufs=2) as nmp,
                tc.tile_pool(name="rcp", bufs=2) as rcp,
                tc.tile_pool(name="scp", bufs=2) as scp,
                tc.tile_pool(name="svp", bufs=2) as svp,
                tc.tile_pool(name="obp", bufs=2) as obp,
            ):
                hf_prev = hb_prev = None
                for s in range(NT if STAGE > 1 else 0):
                    new_state = {}
                    for dire, (hpool, h_prev) in (("f", (hfp, hf_prev)),
                                                  ("b", (hbp, hb_prev))):
                        t_d = s if dire == "f" else NT - 1 - s
                        pin = pinp.tile([P, ROWS, NY], f32, tag="pin")
                        nc.sync.dma_start(out=pin[:], in_=predr[t_d])
                        h_next = hpool.tile([P, TR, YP], f16, tag=f"h{dire}")
                        if STAGE == 2:
                            nc.vector.memset(h_next[:, 0, :], 0.0)
                            nc.vector.memset(h_next[:, TR - 1, :], 0.0)
                        for r0, nr in SCAN_GROUPS:
                            if s > 0:
                                ps = cps.tile([P, 3, NY], f32, tag="cv")
                                for tap in range(9):
                                    dxi, dyi = divmod(tap, 3)
                                    nc.tensor.matmul(
                                        out=ps[:, :nr, :],
                                        lhsT=wh[tap][:],
                                        rhs=h_prev[:, r0 + dxi:r0 + dxi + nr,
                                                   dyi:dyi + NY],
                                        start=(tap == 0), stop=(tap == 8),
                                    )
                                z = zp.tile([P, 3, NY], f32, tag="z")
                                nc.vector.tensor_tensor(
                                    out=z[:, :nr, :], in0=ps[:, :nr, :],
                                    in1=pin[:, r0:r0 + nr, :], op=ALU.add)
                                zv = z[:, :nr, :]
                            else:
                                zv = pin[:, r0:r0 + nr, :]
                            z2 = z2p.tile([P, 3, NY], f16, tag="z2")
                            nc.scalar.activation(z2[:, :nr, :], zv, AF.Square, scale=0.015625)
                            mp = mps.tile([P, 3, NY], f32, tag="mg")
                            nc.tensor.matmul(out=mp[:, :nr, :], lhsT=pm[:],
                                             rhs=z2[:, :nr, :],
                                             start=True, stop=True)
                            mag = mgp.tile([P, 3, NY], f32, tag="mag")
                            nc.scalar.activation(mag[:, :nr, :], mp[:, :nr, :],
                                                 AF.Sqrt, bias=epst[:, 0:1],
                                                 scale=4096.0)
                            num = nmp.tile([P, 3, NY], f32, tag="num")
                            nc.scalar.activation(num[:, :nr, :], mag[:, :nr, :],
                                                 AF.Relu, bias=bm[:, 0:1])
                            rec = rcp.tile([P, 3, NY], f32, tag="rec")
                            nc.vector.reciprocal_approx_fast(out=rec[:, :nr, :], in_=mag[:, :nr, :])
                            sc = scp.tile([P, 3, NY], f32, tag="sc")
                            nc.vector.tensor_tensor(out=sc[:, :nr, :],
                                                    in0=num[:, :nr, :],
                                                    in1=rec[:, :nr, :],
                                                    op=ALU.mult)
                            nc.vector.tensor_tensor(
                                out=h_next[:, r0 + 1:r0 + 1 + nr, 1:1 + NY],
                                in0=zv, in1=sc[:, :nr, :], op=ALU.mult)
                        if s < NT - 1:
                            # y-edge reflect on owned rows
                            nc.vector.tensor_copy(
                                out=h_next[:, 1:1 + ROWS, 0:1],
                                in_=h_next[:, 1:1 + ROWS, 2:3])
                            nc.vector.tensor_copy(
                                out=h_next[:, 1:1 + ROWS, YP - 1:YP],
                                in_=h_next[:, 1:1 + ROWS, YP - 3:YP - 2])
                        new_state[dire] = h_next

                    hf_next, hb_next = new_state["f"], new_state["b"]

                    if s < NT - 1 and STAGE >= 3:
                        # halo exchange: contribute boundary rows, AllGather,
                        # per-core indirect gather back into halo rows
                        cc_in = dram.tile([NSLOT * P, YP], f16, tag="cci")
                        cc_out = dram.tile([NC_CORES * NSLOT * P, YP], f16,
                                           addr_space="Shared", tag="cco")
                        cci_v = cc_in[:].rearrange("(s p) y -> s p y", p=P)
                        for di, h_n in ((0, hf_next), (1, hb_next)):
                            for si, row in enumerate(CC_ROWS):
                                nc.sync.dma_start(out=cci_v[di * 4 + si],
                                                  in_=h_n[:, row, :])
                        if STAGE != 5:
                            nc.gpsimd.collective_compute(
                                "AllGather", ALU.bypass,
                                replica_groups=[list(range(NC_CORES))],
                                ins=[cc_in[:].opt()], outs=[cc_out[:].opt()],
                            )
                        else:
                            nc.sync.dma_start(out=cc_out[:NSLOT * P, :],
                                              in_=cc_in[:])
                        cco_v = cc_out[:].rearrange("(r s p) y -> r s p y",
                                                    s=NSLOT, p=P)
                        if STAGE == 3:
                            # consume cc_out with fixed slices (wrong data, AG live)
                            nc.sync.dma_start(out=hf_next[:, 0, :], in_=cco_v[0, 1])
                            nc.sync.dma_start(out=hf_next[:, TR - 1, :], in_=cco_v[0, 2])
                            nc.sync.dma_start(out=hb_next[:, 0, :], in_=cco_v[0, 5])
                            nc.sync.dma_start(out=hb_next[:, TR - 1, :], in_=cco_v[0, 6])
                        else:
                            for k, (h_n, row) in enumerate(
                                    ((hf_next, 0), (hf_next, TR - 1),
                                     (hb_next, 0), (hb_next, TR - 1))):
                                nc.gpsimd.indirect_dma_start(
                                    out=h_n[:, row, :], out_offset=None,
                                    in_=cc_out[:],
                                    in_offset=bass.IndirectOffsetOnAxis(
                                        ap=hix[:, k:k + 1], axis=0),
                                )

                    # save / combine
                    if s <= 5:
                        nc.sync.dma_start(
                            out=sav[s],
                            in_=hf_next[:, 1:1 + ROWS, 1:1 + NY])
                        nc.sync.dma_start(
                            out=sav[NT - 1 - s],
                            in_=hb_next[:, 1:1 + ROWS, 1:1 + NY])
                    else:
                        for h_n, t_o in ((hf_next, s), (hb_next, NT - 1 - s)):
                            ld = svp.tile([P, ROWS, NY], f16, tag="ld")
                            nc.sync.dma_start(out=ld[:], in_=sav[t_o])
                            ob = obp.tile([P, ROWS, NY], f32, tag="ob")
                            nc.vector.tensor_tensor(
                                out=ob[:], in0=h_n[:, 1:1 + ROWS, 1:1 + NY],
                                in1=ld[:], op=ALU.add)
                            nc.sync.dma_start(out=out[t_o], in_=ob[:])

                    hf_prev, hb_prev = hf_next, hb_next

    nc.compile()
    _CACHED["nc"] = nc
    return nc


def _complex_lhsT(wr, wi_):
    """[O=64, I=64or2, 3, 3] complex pair -> per-tap lhsT [9, 2*I, 128]."""
    O, I = wr.shape[:2]
    lhsT = np.zeros((9, 2 * I, 2 * O), np.float32)
    for tap in range(9):
        kx, ky = divmod(tap, 3)
        lhsT[tap, :I, :O] = wr[:, :, kx, ky].T
        lhsT[tap, I:, :O] = -wi_[:, :, kx, ky].T
        lhsT[tap, :I, O:] = wi_[:, :, kx, ky].T
        lhsT[tap, I:, O:] = wr[:, :, kx, ky].T
    return lhsT


def kernel(**inputs):
    inp_r = np.asarray(inputs["input_r"], np.float32)
    inp_i = np.asarray(inputs["input_i"], np.float32)
    itr_r = np.asarray(inputs["iter_r"], np.float32)
    itr_i = np.asarray(inputs["iter_i"], np.float32)

    # ---- weights ----
    wh2h = _complex_lhsT(np.asarray(inputs["w_h2h_r"]), np.asarray(inputs["w_h2h_i"]))
    wih = _complex_lhsT(np.asarray(inputs["w_ih_r"]), np.asarray(inputs["w_ih_i"]))
    w4 = _complex_lhsT(np.asarray(inputs["w_i2h_r"]), np.asarray(inputs["w_i2h_i"]))
    # i2h as K=36 im2col weight: k = tap*4 + cin4
    wi2h = np.ascontiguousarray(
        w4.reshape(9, 4, P).reshape(36, P))
    pmat = np.zeros((P, P), np.float32)
    for k in range(P):
        pmat[k, k % HID] = 1.0
        pmat[k, HID + k % HID] = 1.0
    b_pre = np.concatenate([
        inputs["b_i2h_r"] + inputs["b_ih_r"] + inputs["b_h2h_r"],
        inputs["b_i2h_i"] + inputs["b_ih_i"] + inputs["b_h2h_i"],
    ]).astype(np.float32)[:, None]
    b_mod = np.tile(np.asarray(inputs["mod_b"], np.float32), 2)[:, None]

    # ---- activations, reflect-padded [t, ch, xpad, ypad] ----
    itg = np.concatenate([itr_r[0], itr_i[0]], axis=0).transpose(1, 0, 2, 3)
    itg = np.pad(itg, ((0, 0), (0, 0), (1, 1), (1, 1)), mode="reflect")
    ing = np.concatenate([inp_r[0], inp_i[0]], axis=0).transpose(1, 0, 2, 3)
    ing = np.pad(ing, ((0, 0), (0, 0), (1, 1), (1, 1)), mode="reflect")

    in_maps = []
    for c in range(NC_CORES):
        a = c * ROWS
        iter_il = np.ascontiguousarray(itg[:, :, a:a + TR, :])
        im2col = np.empty((NT, 36, ROWS, NY), np.float32)
        for tap in range(9):
            kx, ky = divmod(tap, 3)
            for c4 in range(4):
                im2col[:, tap * 4 + c4] = ing[:, c4, a + kx:a + kx + ROWS,
                                              ky:ky + NY]
        hidx = np.zeros((P, 4), np.int32)
        pa = np.arange(P)

        def flat(rank, slot):
            return (rank * NSLOT + slot) * P + pa

        for base, dirs in ((0, 0), (2, 4)):
            # top halo (k=base): reflect for core 0 else left nbr last row
            hidx[:, base] = flat(0, dirs + 1) if c == 0 else flat(c - 1, dirs + 3)
            # bottom halo: reflect for core 7 else right nbr first row
            hidx[:, base + 1] = (flat(NC_CORES - 1, dirs + 2) if c == NC_CORES - 1
                                 else flat(c + 1, dirs + 0))
        in_maps.append({
            "iter_il": iter_il.astype(np.float16),
            "im2col": im2col.astype(np.float16),
            "wh2h": wh2h.astype(np.float16), "wih": wih.astype(np.float16),
            "wi2h": wi2h.astype(np.float16), "pmat": pmat.astype(np.float16),
            "b_pre": b_pre, "b_mod": b_mod, "hidx": hidx,
        })

    nc = _build()
    res = run_bass_kernel_spmd(nc, in_maps, core_ids=list(range(NC_CORES)),
                               trace=TRACE)
    LAST["exec_time_ns"] = res.exec_time_ns
    LAST["results"] = res

    full = np.empty((1, HID, NT, NX, NY, 2), np.float32)
    for c in range(NC_CORES):
        a = c * ROWS
        o = res.results[c]["out"]          # [NT, 128, ROWS, NY]
        full[0, :, :, a:a + ROWS, :, 0] = o[:, :HID].transpose(1, 0, 2, 3)
        full[0, :, :, a:a + ROWS, :, 1] = o[:, HID:].transpose(1, 0, 2, 3)
    return full
